# revision 10
# baseline (speedup 1.0000x reference)
"""DocRED relation-extraction head on 8 Trainium2 NeuronCores.

Data-parallel over the batch axis: core b owns batch b's hidden_states slab
and its entity/pair indices; the classifier weights are replicated.

Pipeline (v3):
  - pos leads the SYNC ring (before the W-slab flood) so its completion
    semaphore fires ~8.3us and the indirect gather can start immediately;
    small constants travel as ONE merged DMA per ring to avoid sem-lane
    reuse stalls.
  - stage B runs with the dense_w slab as the STATIONARY operand in
    [128h, 128j] chunks and repT as the 32-col moving operand, so proj
    comes out of PSUM already j-on-partitions (projT) - no transposes.
  - eL1 interleaves between the two stage-B halves (PSUM slot order makes
    it reuse a freed half-0 bank).
  - PE warmup bridges the preamble->gather window so HAM stays at 8/8.
  - everything travels fp16; PSUM accumulation is fp32. ~7e-4 rel err.

    repT    = mention-sum of 128 hidden rows via indirect-DMA gather +
              8 matmuls against a block-ones matrix (fuses sum + transpose)
    projT   = per half: 8 PSUM banks accumulate lhsT=dw[128h,128j] chunks
              over the 8 h-chunks (W is the dominant 4MB DMA stream)
    eL1'    = [projT1 | dense_b].T @ ow  [33, 98]  (row 32 = const row)
    eL2     = projT2.T @ ow              [32, 98]
    logits[p] = eL1'[head[p]] + const + eL2[tail[p]], via ONE K=65-stacked
              one-hot matmul per 128-pair tile.
"""

import numpy as np
from contextlib import ExitStack

import concourse.bass as bass
import concourse.bacc as bacc
import concourse.tile as tile
import concourse.mybir as mybir
from concourse.bass_utils import run_bass_kernel_spmd

B, L, H, E, M, P, C = 8, 2048, 1024, 32, 4, 1024, 97
N_CORES = 8
HC = H // 128   # h-chunks per half (contraction of dense)
JC = H // 128   # j-chunks (output of dense / contraction of out proj)
PT = P // 128   # pair tiles
SLOT = E + 1    # projT slot width: 32 cols projT + 1 col dense_b chunk
CP = C + 1      # class dim padded to 98 (alignment; pad column zero)

f32 = mybir.dt.float32
f16 = mybir.dt.float16
i32 = mybir.dt.int32

N_WARM = 44     # f32 warmup matmul pairs bridging preamble->gather window

# merged constant tensor "misc" column layout (all fp16, 128 partitions)
ONES0 = 0                 # [128, 32] block-ones for the mention sum
DB0 = ONES0 + E           # [128, 8] dense_b chunks
IOTA0 = DB0 + HC          # [32, 1] iota column
OB0 = IOTA0 + 1           # [1, 98] out_b on row 0 (zero padded)
OW0 = OB0 + CP            # [128, 8*98] out_w chunks
MISCW = OW0 + JC * CP

_CACHE = {}


def _build():
    nc = bacc.Bacc("TRN2", target_bir_lowering=False, debug=False)

    hs2 = [nc.dram_tensor(f"hs{gh}", [L, H // 2], f16, kind="ExternalInput").ap()
           for gh in range(2)]
    pos = nc.dram_tensor("pos", [E * M, 1], i32, kind="ExternalInput").ap()
    misc = nc.dram_tensor("misc", [128, MISCW], f16, kind="ExternalInput").ap()
    hrtr = nc.dram_tensor("hrtr", [E, 2 * P], f16, kind="ExternalInput").ap()
    dw = nc.dram_tensor("dw", [2 * H, H], f16, kind="ExternalInput").ap()
    # output laid out [128, PT*C] fp16: pair-tile t in columns t*C..(t+1)*C;
    # host upcasts + reshapes to [P, C]
    out = nc.dram_tensor("out", [128, PT * C], f16, kind="ExternalOutput").ap()

    with tile.TileContext(nc) as tc, ExitStack() as ctx:
        sb = ctx.enter_context(tc.tile_pool(name="sb", bufs=1))
        wpool = ctx.enter_context(tc.tile_pool(name="w", bufs=16))
        pspool = ctx.enter_context(tc.tile_pool(name="ps", bufs=8, space="PSUM"))

        # ---- pos alone at the head of the sync ring: its completion sem
        # gates the gather, so it must clear before the W flood.
        sb_pos = sb.tile([E * M, 1], i32)
        nc.sync.dma_start(sb_pos[:], pos[:])

        # ---- gather the 128 mention rows of hidden_states (SWDGE), split
        # into two column-halves so stage A/B can start on h-chunks 0-3
        # while the second half is still in flight.
        sb_g = sb.tile([E * M, H], f16)
        for gh in range(2):
            nc.gpsimd.indirect_dma_start(
                out=sb_g[:, gh * 512:(gh + 1) * 512],
                out_offset=None,
                in_=hs2[gh][:],
                in_offset=bass.IndirectOffsetOnAxis(ap=sb_pos[:, :1], axis=0),
            )

        # ---- merged constants on the scalar ring; W slabs alternate
        # between the two HWDGE rings (even->sync, odd->scalar).
        sb_misc = sb.tile([128, MISCW], f16)
        nc.scalar.dma_start(sb_misc[:], misc[:])
        wt = []
        for s in range(2 * HC):
            wt.append(wpool.tile([128, H], f16, tag="wslab", name=f"wt{s}"))
        nc.scalar.dma_start(wt[1][:], dw[128:256, :])
        sb_hrtr = sb.tile([E, 2 * P], f16)
        nc.scalar.dma_start(sb_hrtr[:], hrtr[:])
        for s in range(3, 2 * HC, 2):
            nc.scalar.dma_start(wt[s][:], dw[s * 128:(s + 1) * 128, :])
        for s in range(0, 2 * HC, 2):
            nc.sync.dma_start(wt[s][:], dw[s * 128:(s + 1) * 128, :])

        # ---- PE warm-up: HAM needs ~3.4us of sustained activity to release
        # the clock gate. Sized to end about when the gather lands.
        wdum = sb.tile([128, E], f32)
        nc.vector.memset(wdum[:], 0.0)
        ps_warm = pspool.tile([E, E], f32, tag="ps")
        for i in range(N_WARM):
            nc.tensor.matmul(
                out=ps_warm[:], lhsT=wdum[:], rhs=wdum[:],
                start=True, stop=True,
            )
        # slab-paced blip matmuls: one tiny matmul gated on each early W
        # slab's arrival keeps HAM from re-throttling between the dense
        # warmup and the gather-gated stage A, without clogging the PE.
        for s in range(2, HC):
            nc.tensor.matmul(
                out=ps_warm[:], lhsT=wt[s][:, :E], rhs=wt[s][:, :E],
                start=True, stop=True,
            )

        # ---- one-hot pair operands (DVE, early - only needs hrtr/iota)
        sb_oh = sb.tile([2 * E + 1, P], f16)
        nc.vector.tensor_tensor(
            out=sb_oh[:E, :],
            in0=sb_misc[:E, IOTA0:IOTA0 + 1].to_broadcast([E, P]),
            in1=sb_hrtr[:, :P],
            op=mybir.AluOpType.is_equal,
        )
        nc.vector.tensor_tensor(
            out=sb_oh[E:2 * E, :],
            in0=sb_misc[:E, IOTA0:IOTA0 + 1].to_broadcast([E, P]),
            in1=sb_hrtr[:, P:],
            op=mybir.AluOpType.is_equal,
        )
        nc.vector.tensor_tensor(
            out=sb_oh[2 * E:2 * E + 1, :],
            in0=sb_misc[:1, IOTA0:IOTA0 + 1].to_broadcast([1, P]),
            in1=sb_misc[:1, IOTA0:IOTA0 + 1].to_broadcast([1, P]),
            op=mybir.AluOpType.is_equal,
        )

        # ---- projT slot buffer; dense_b chunks ride as col 32 of half-0
        # slots so the const row falls out of the eL1 matmul
        sb_projT = sb.tile([128, 2 * JC * SLOT], f16)
        for jc in range(JC):
            nc.vector.tensor_copy(
                out=sb_projT[:, jc * SLOT + E:jc * SLOT + E + 1],
                in_=sb_misc[:, DB0 + jc:DB0 + jc + 1],
            )

        # ---- stage A: repT[h, e] = sum_m gathered[4e+m, h], one [128,32]
        # PSUM per h-chunk, drained to fp16
        sb_repT = sb.tile([128, HC * E], f16)
        for hc in range(HC):
            pa = pspool.tile([128, E], f32, tag="ps", name=f"pa{hc}")
            nc.tensor.matmul(
                out=pa[:],
                lhsT=sb_g[:, hc * 128:(hc + 1) * 128],
                rhs=sb_misc[:, ONES0:ONES0 + E],
                start=True, stop=True,
            )
            nc.vector.tensor_copy(out=sb_repT[:, hc * E:(hc + 1) * E], in_=pa[:])

        # ---- stage B: projT[j, e] accumulated over h-chunks with the W slab
        # chunk as the stationary operand -> output is j-on-partitions.
        # eL1 is emitted between the halves: its PSUM tile reuses a freed
        # half-0 slot and its matmuls slot into the half-1 DMA-wait gaps.
        ps_eL = []
        for half in range(2):
            ps_b = [pspool.tile([128, E], f32, tag="ps", name=f"psb{half}_{jc}")
                    for jc in range(JC)]
            for hc in range(HC):
                s = half * HC + hc
                for jc in range(JC):
                    nc.tensor.matmul(
                        out=ps_b[jc][:],
                        lhsT=wt[s][:, jc * 128:(jc + 1) * 128],
                        rhs=sb_repT[:, hc * E:(hc + 1) * E],
                        start=(hc == 0),
                        stop=(hc == HC - 1),
                    )
            for jc in range(JC):
                slot = (half * JC + jc) * SLOT
                nc.vector.tensor_copy(
                    out=sb_projT[:, slot:slot + E], in_=ps_b[jc][:])
            # eL for this half: half-0 lhsT is 33 wide (dense_b col)
            w_m = SLOT if half == 0 else E
            eL = pspool.tile([w_m, CP], f32, tag="ps", name=f"eL{half}")
            ps_eL.append(eL)
            for jc in range(JC):
                slot = (half * JC + jc) * SLOT
                nc.tensor.matmul(
                    out=eL[:],
                    lhsT=sb_projT[:, slot:slot + w_m],
                    rhs=sb_misc[:, OW0 + jc * CP:OW0 + (jc + 1) * CP],
                    start=(jc == 0), stop=(jc == JC - 1),
                )

        # ---- eL stack [65, 98]: rows 0-31 eL1, 32-63 eL2,
        # row 64 = dense_b @ ow + out_b
        sb_eL = sb.tile([2 * E + 1, CP], f16)
        nc.vector.tensor_copy(out=sb_eL[:E, :], in_=ps_eL[0][:E, :])
        nc.vector.tensor_copy(out=sb_eL[E:2 * E, :], in_=ps_eL[1][:])
        nc.vector.tensor_add(
            out=sb_eL[2 * E:2 * E + 1, :], in0=ps_eL[0][E:E + 1, :],
            in1=sb_misc[:1, OB0:OB0 + CP])

        # ---- stage D: stacked one-hot pair gather, one matmul per 128 pairs
        sb_out = sb.tile([128, PT * C], f16)
        for pt in range(PT):
            pl = pspool.tile([128, CP], f32, tag="ps", name=f"pl{pt}")
            nc.tensor.matmul(
                out=pl[:],
                lhsT=sb_oh[:, pt * 128:(pt + 1) * 128],
                rhs=sb_eL[:],
                start=True, stop=True,
            )
            nc.vector.tensor_copy(
                out=sb_out[:, pt * C:(pt + 1) * C], in_=pl[:, :C])
            if pt == PT // 2 - 1:
                nc.scalar.dma_start(
                    out[:, :PT * C // 2], sb_out[:, :PT * C // 2])
        nc.sync.dma_start(out[:, PT * C // 2:], sb_out[:, PT * C // 2:])

    nc.compile()
    return nc


def get_compiled():
    if "nc" not in _CACHE:
        _CACHE["nc"] = _build()
    return _CACHE["nc"]


def make_in_maps(hidden_states, dense_w, dense_b, out_w, out_b,
                 entity_position_ids, head_tail_idxs):
    hidden_states = np.asarray(hidden_states)
    dense_w = np.asarray(dense_w)
    dense_b = np.asarray(dense_b)
    out_w = np.asarray(out_w)
    out_b = np.asarray(out_b)
    entity_position_ids = np.asarray(entity_position_ids)
    head_tail_idxs = np.asarray(head_tail_idxs)

    misc = np.zeros((128, MISCW), np.float16)
    misc[:, ONES0:ONES0 + E] = np.repeat(np.eye(E, dtype=np.float16), M, axis=0)
    misc[:, DB0:DB0 + HC] = (
        np.asarray(dense_b, np.float16).reshape(HC, 128).T)
    misc[:E, IOTA0] = np.arange(E, dtype=np.float16)
    misc[0, OB0:OB0 + C] = np.asarray(out_b, np.float16)  # col 97 stays 0
    owp = np.zeros((H, CP), np.float16)
    owp[:, :C] = np.asarray(out_w, np.float16)
    misc[:, OW0:] = (
        owp.reshape(JC, 128, CP).transpose(1, 0, 2).reshape(128, JC * CP))

    dense_w = np.ascontiguousarray(dense_w, dtype=np.float16)
    in_maps = []
    for b in range(B):
        ht = head_tail_idxs[b].astype(np.float16)  # [P, 2]
        hrtr = np.empty((E, 2 * P), np.float16)
        hrtr[:, :P] = ht[None, :, 0]
        hrtr[:, P:] = ht[None, :, 1]
        hsb = np.asarray(hidden_states[b], dtype=np.float16)
        in_maps.append({
            "hs0": np.ascontiguousarray(hsb[:, :H // 2]),
            "hs1": np.ascontiguousarray(hsb[:, H // 2:]),
            "pos": np.ascontiguousarray(
                entity_position_ids[b].reshape(E * M, 1).astype(np.int32)),
            "misc": misc,
            "hrtr": hrtr,
            "dw": dense_w,
        })
    return in_maps


def kernel(hidden_states, dense_w, dense_b, out_w, out_b,
           entity_position_ids, head_tail_idxs, _trace=False, _trace_kwargs=None):
    nc = get_compiled()
    in_maps = make_in_maps(hidden_states, dense_w, dense_b, out_w, out_b,
                           entity_position_ids, head_tail_idxs)
    res = run_bass_kernel_spmd(
        nc, in_maps, core_ids=list(range(N_CORES)),
        trace=_trace, **(_trace_kwargs or {}),
    )
    outp = np.concatenate(
        [res.results[i]["out"].astype(np.float32)
         .reshape(128, PT, C).transpose(1, 0, 2).reshape(P, C)
         for i in range(N_CORES)], axis=0)
    if _trace:
        return outp, res
    return outp


# revision 14
# speedup vs baseline: 1.1972x; 1.1972x over previous
"""DocRED relation-extraction head on 8 Trainium2 NeuronCores.

Data-parallel over the batch axis: core b owns batch b's hidden_states slab
and its entity/pair indices; the classifier weights are replicated.

Pipeline (v3):
  - pos leads the SYNC ring (before the W-slab flood) so its completion
    semaphore fires ~8.3us and the indirect gather can start immediately;
    small constants travel as ONE merged DMA per ring to avoid sem-lane
    reuse stalls.
  - stage B runs with the dense_w slab as the STATIONARY operand in
    [128h, 128j] chunks and repT as the 32-col moving operand, so proj
    comes out of PSUM already j-on-partitions (projT) - no transposes.
  - eL1 interleaves between the two stage-B halves (PSUM slot order makes
    it reuse a freed half-0 bank).
  - PE warmup bridges the preamble->gather window so HAM stays at 8/8.
  - everything travels fp16; PSUM accumulation is fp32. ~7e-4 rel err.

    repT    = mention-sum of 128 hidden rows via indirect-DMA gather +
              8 matmuls against a block-ones matrix (fuses sum + transpose)
    projT   = per half: 8 PSUM banks accumulate lhsT=dw[128h,128j] chunks
              over the 8 h-chunks (W is the dominant 4MB DMA stream)
    eL1'    = [projT1 | dense_b].T @ ow  [33, 98]  (row 32 = const row)
    eL2     = projT2.T @ ow              [32, 98]
    logits[p] = eL1'[head[p]] + const + eL2[tail[p]], via ONE K=65-stacked
              one-hot matmul per 128-pair tile.
"""

import numpy as np
from contextlib import ExitStack

import concourse.bass as bass
import concourse.bacc as bacc
import concourse.tile as tile
import concourse.mybir as mybir
from concourse.bass_utils import run_bass_kernel_spmd

B, L, H, E, M, P, C = 8, 2048, 1024, 32, 4, 1024, 97
N_CORES = 8
HC = H // 128   # h-chunks per half (contraction of dense)
JC = H // 128   # j-chunks (output of dense / contraction of out proj)
PT = P // 128   # pair tiles
SLOT = E + 1    # projT slot width: 32 cols projT + 1 col dense_b chunk
CP = C + 1      # class dim padded to 98 (alignment; pad column zero)

f32 = mybir.dt.float32
f16 = mybir.dt.float16
i32 = mybir.dt.int32

N_WARM = 44     # f32 warmup matmul pairs bridging preamble->gather window

# merged constant tensor "misc" column layout (all fp16, 128 partitions)
ONES0 = 0                 # [128, 32] block-ones for the mention sum
DB0 = ONES0 + E           # [128, 8] dense_b chunks
IOTA0 = DB0 + HC          # [32, 1] iota column
OB0 = IOTA0 + 1           # [1, 98] out_b on row 0 (zero padded)
OW0 = OB0 + CP            # [128, 8*98] out_w chunks
MISCW = OW0 + JC * CP

_CACHE = {}


def _build():
    nc = bacc.Bacc("TRN2", target_bir_lowering=False, debug=False)

    hs = nc.dram_tensor("hs", [L, H], f16, kind="ExternalInput").ap()
    pos = nc.dram_tensor("pos", [E * M, 1], i32, kind="ExternalInput").ap()
    misc = nc.dram_tensor("misc", [128, MISCW], f16, kind="ExternalInput").ap()
    hrtr = nc.dram_tensor("hrtr", [E, 2 * P], f16, kind="ExternalInput").ap()
    dw = nc.dram_tensor("dw", [2 * H, H], f16, kind="ExternalInput").ap()
    # output laid out [128, PT*C] fp16: pair-tile t in columns t*C..(t+1)*C;
    # host upcasts + reshapes to [P, C]
    out = nc.dram_tensor("out", [128, PT * C], f16, kind="ExternalOutput").ap()

    with tile.TileContext(nc) as tc, ExitStack() as ctx:
        sb = ctx.enter_context(tc.tile_pool(name="sb", bufs=1))
        wpool = ctx.enter_context(tc.tile_pool(name="w", bufs=16))
        pspool = ctx.enter_context(tc.tile_pool(name="ps", bufs=8, space="PSUM"))

        # ---- pos alone at the head of the sync ring: its completion sem
        # gates the gather, so it must clear before the W flood.
        sb_pos = sb.tile([E * M, 1], i32)
        nc.sync.dma_start(sb_pos[:], pos[:])

        # ---- gather the 128 mention rows of hidden_states (SWDGE lanes are
        # separate from the HWDGE lanes, so this never stalls slab issues)
        sb_g = sb.tile([E * M, H], f16)
        nc.gpsimd.indirect_dma_start(
            out=sb_g[:],
            out_offset=None,
            in_=hs[:],
            in_offset=bass.IndirectOffsetOnAxis(ap=sb_pos[:, :1], axis=0),
        )

        # ---- merged constants on the scalar ring; W slabs alternate
        # between the two HWDGE rings (even->sync, odd->scalar).
        # EMISSION ORDER = HWDGE sem-lane order (global round-robin of 8):
        # pos,misc,hrtr,s0..s4 are gen-1; s5..s12 reuse lanes of
        # early-completing DMAs; all reuse is benign.
        sb_misc = sb.tile([128, MISCW], f16)
        nc.scalar.dma_start(sb_misc[:], misc[:])
        sb_hrtr = sb.tile([E, 2 * P], f16)
        nc.scalar.dma_start(sb_hrtr[:], hrtr[:])
        wt = []
        for s in range(2 * HC):
            wt.append(wpool.tile([128, H], f16, tag="wslab", name=f"wt{s}"))
        for s in range(2 * HC):
            eng = nc.sync if s % 2 == 0 else nc.scalar
            eng.dma_start(wt[s][:], dw[s * 128:(s + 1) * 128, :])

        # ---- PE warm-up: HAM needs ~3.4us of sustained activity to release
        # the clock gate. Sized to end about when the gather lands.
        wdum = sb.tile([128, E], f32)
        nc.vector.memset(wdum[:], 0.0)
        ps_warm = pspool.tile([E, E], f32, tag="ps")
        for i in range(N_WARM):
            nc.tensor.matmul(
                out=ps_warm[:], lhsT=wdum[:], rhs=wdum[:],
                start=True, stop=True,
            )
        # slab-paced blip matmuls: one tiny matmul gated on each early W
        # slab's arrival keeps HAM from re-throttling between the dense
        # warmup and the gather-gated stage A, without clogging the PE.
        for s in range(1, 5):
            nc.tensor.matmul(
                out=ps_warm[:], lhsT=wt[s][:, :E], rhs=wt[s][:, :E],
                start=True, stop=True,
            )

        # ---- one-hot pair operands (DVE, early - only needs hrtr/iota)
        sb_oh = sb.tile([2 * E + 1, P], f16)
        nc.vector.tensor_tensor(
            out=sb_oh[:E, :],
            in0=sb_misc[:E, IOTA0:IOTA0 + 1].to_broadcast([E, P]),
            in1=sb_hrtr[:, :P],
            op=mybir.AluOpType.is_equal,
        )
        nc.vector.tensor_tensor(
            out=sb_oh[E:2 * E, :],
            in0=sb_misc[:E, IOTA0:IOTA0 + 1].to_broadcast([E, P]),
            in1=sb_hrtr[:, P:],
            op=mybir.AluOpType.is_equal,
        )
        nc.vector.tensor_tensor(
            out=sb_oh[2 * E:2 * E + 1, :],
            in0=sb_misc[:1, IOTA0:IOTA0 + 1].to_broadcast([1, P]),
            in1=sb_misc[:1, IOTA0:IOTA0 + 1].to_broadcast([1, P]),
            op=mybir.AluOpType.is_equal,
        )

        # ---- projT slot buffer; dense_b chunks ride as col 32 of half-0
        # slots so the const row falls out of the eL1 matmul
        sb_projT = sb.tile([128, 2 * JC * SLOT], f16)
        for jc in range(JC):
            nc.vector.tensor_copy(
                out=sb_projT[:, jc * SLOT + E:jc * SLOT + E + 1],
                in_=sb_misc[:, DB0 + jc:DB0 + jc + 1],
            )

        # ---- stage A: repT[h, e] = sum_m gathered[4e+m, h], one [128,32]
        # PSUM per h-chunk, drained to fp16
        sb_repT = sb.tile([128, HC * E], f16)
        for hc in range(HC):
            pa = pspool.tile([128, E], f32, tag="ps", name=f"pa{hc}")
            nc.tensor.matmul(
                out=pa[:],
                lhsT=sb_g[:, hc * 128:(hc + 1) * 128],
                rhs=sb_misc[:, ONES0:ONES0 + E],
                start=True, stop=True,
            )
            nc.vector.tensor_copy(out=sb_repT[:, hc * E:(hc + 1) * E], in_=pa[:])

        # ---- stage B: projT[j, e] accumulated over h-chunks with the W slab
        # chunk as the stationary operand -> output is j-on-partitions.
        # eL1 is emitted between the halves: its PSUM tile reuses a freed
        # half-0 slot and its matmuls slot into the half-1 DMA-wait gaps.
        ps_eL = []
        for half in range(2):
            ps_b = [pspool.tile([128, E], f32, tag="ps", name=f"psb{half}_{jc}")
                    for jc in range(JC)]
            for hc in range(HC):
                s = half * HC + hc
                for jc in range(JC):
                    nc.tensor.matmul(
                        out=ps_b[jc][:],
                        lhsT=wt[s][:, jc * 128:(jc + 1) * 128],
                        rhs=sb_repT[:, hc * E:(hc + 1) * E],
                        start=(hc == 0),
                        stop=(hc == HC - 1),
                    )
            for jc in range(JC):
                slot = (half * JC + jc) * SLOT
                nc.vector.tensor_copy(
                    out=sb_projT[:, slot:slot + E], in_=ps_b[jc][:])
            # eL for this half: half-0 lhsT is 33 wide (dense_b col)
            w_m = SLOT if half == 0 else E
            eL = pspool.tile([w_m, CP], f32, tag="ps", name=f"eL{half}")
            ps_eL.append(eL)
            for jc in range(JC):
                slot = (half * JC + jc) * SLOT
                nc.tensor.matmul(
                    out=eL[:],
                    lhsT=sb_projT[:, slot:slot + w_m],
                    rhs=sb_misc[:, OW0 + jc * CP:OW0 + (jc + 1) * CP],
                    start=(jc == 0), stop=(jc == JC - 1),
                )

        # ---- eL stack [65, 98]: rows 0-31 eL1, 32-63 eL2,
        # row 64 = dense_b @ ow + out_b
        sb_eL = sb.tile([2 * E + 1, CP], f16)
        nc.vector.tensor_copy(out=sb_eL[:E, :], in_=ps_eL[0][:E, :])
        nc.vector.tensor_copy(out=sb_eL[E:2 * E, :], in_=ps_eL[1][:])
        nc.vector.tensor_add(
            out=sb_eL[2 * E:2 * E + 1, :], in0=ps_eL[0][E:E + 1, :],
            in1=sb_misc[:1, OB0:OB0 + CP])

        # ---- stage D: stacked one-hot pair gather, one matmul per 128 pairs
        sb_out = sb.tile([128, PT * C], f16)
        for pt in range(PT):
            pl = pspool.tile([128, CP], f32, tag="ps", name=f"pl{pt}")
            nc.tensor.matmul(
                out=pl[:],
                lhsT=sb_oh[:, pt * 128:(pt + 1) * 128],
                rhs=sb_eL[:],
                start=True, stop=True,
            )
            nc.vector.tensor_copy(
                out=sb_out[:, pt * C:(pt + 1) * C], in_=pl[:, :C])
            if pt == PT // 2 - 1:
                nc.scalar.dma_start(
                    out[:, :PT * C // 2], sb_out[:, :PT * C // 2])
        nc.sync.dma_start(out[:, PT * C // 2:], sb_out[:, PT * C // 2:])

    nc.compile()
    return nc


def get_compiled():
    if "nc" not in _CACHE:
        _CACHE["nc"] = _build()
    return _CACHE["nc"]


def make_in_maps(hidden_states, dense_w, dense_b, out_w, out_b,
                 entity_position_ids, head_tail_idxs):
    hidden_states = np.asarray(hidden_states)
    dense_w = np.asarray(dense_w)
    dense_b = np.asarray(dense_b)
    out_w = np.asarray(out_w)
    out_b = np.asarray(out_b)
    entity_position_ids = np.asarray(entity_position_ids)
    head_tail_idxs = np.asarray(head_tail_idxs)

    misc = np.zeros((128, MISCW), np.float16)
    misc[:, ONES0:ONES0 + E] = np.repeat(np.eye(E, dtype=np.float16), M, axis=0)
    misc[:, DB0:DB0 + HC] = (
        np.asarray(dense_b, np.float16).reshape(HC, 128).T)
    misc[:E, IOTA0] = np.arange(E, dtype=np.float16)
    misc[0, OB0:OB0 + C] = np.asarray(out_b, np.float16)  # col 97 stays 0
    owp = np.zeros((H, CP), np.float16)
    owp[:, :C] = np.asarray(out_w, np.float16)
    misc[:, OW0:] = (
        owp.reshape(JC, 128, CP).transpose(1, 0, 2).reshape(128, JC * CP))

    dense_w = np.ascontiguousarray(dense_w, dtype=np.float16)
    in_maps = []
    for b in range(B):
        ht = head_tail_idxs[b].astype(np.float16)  # [P, 2]
        hrtr = np.empty((E, 2 * P), np.float16)
        hrtr[:, :P] = ht[None, :, 0]
        hrtr[:, P:] = ht[None, :, 1]
        in_maps.append({
            "hs": np.ascontiguousarray(hidden_states[b], dtype=np.float16),
            "pos": np.ascontiguousarray(
                entity_position_ids[b].reshape(E * M, 1).astype(np.int32)),
            "misc": misc,
            "hrtr": hrtr,
            "dw": dense_w,
        })
    return in_maps


def kernel(hidden_states, dense_w, dense_b, out_w, out_b,
           entity_position_ids, head_tail_idxs, _trace=False, _trace_kwargs=None):
    nc = get_compiled()
    in_maps = make_in_maps(hidden_states, dense_w, dense_b, out_w, out_b,
                           entity_position_ids, head_tail_idxs)
    res = run_bass_kernel_spmd(
        nc, in_maps, core_ids=list(range(N_CORES)),
        trace=_trace, **(_trace_kwargs or {}),
    )
    outp = np.concatenate(
        [res.results[i]["out"].astype(np.float32)
         .reshape(128, PT, C).transpose(1, 0, 2).reshape(P, C)
         for i in range(N_CORES)], axis=0)
    if _trace:
        return outp, res
    return outp
